# revision 5
# baseline (speedup 1.0000x reference)
"""CropRandomizer (pos_enc=True) Trainium2 kernel.

Full inputs: images [64,3,240,240] f32, crop_inds_h/w [64,8] i32 (0..23).
Full output: [512, 5, 216, 216] f32 (3 img channels + 2 pos channels, 8
random 216x216 crops per image).

Strategy (data-parallel over 8 NeuronCores, 8 images per core):
- Host prepends the two positional-encoding planes (constant meshgrid) to
  each image -> per-core src [8, 5, 240, 240] in DRAM.
- Each crop is one DRAM->DRAM DMA: out[k] (contiguous [5,216,216]) <-
  src[b, :, h0:h0+216, w0:w0+216].  No SBUF staging, so the only DMA
  payload is the output itself (59.7MB/core at the ~360GB/s DMA roofline).
- h0/w0 are read into sequencer registers straight from the DRAM offset
  table (values_load), so one compiled program serves all cores.
- Raw bass (no TileContext): the 64 crop DMAs are fully independent
  (read-only DRAM in, disjoint DRAM out), alternating SP/Activation HWDGE
  queues; each bumps a per-queue completion semaphore (DMA sems count in
  units of 16) and the program ends with wait_ge on both, skipping the
  tile framework's exit drain/barrier sequence.
"""
import numpy as np

import concourse.bacc as bacc
import concourse.bass as bass
import concourse.mybir as mybir
from concourse.bass import ds
from concourse.bass_utils import run_bass_kernel_spmd

H = W = 240
CROP = 216
B_PER_CORE = 8
N_CROPS = 8
CP = 5
N_CORES = 8
MAX_OFF = H - CROP - 1

_PROGRAM = None


def _build_program():
    nc = bacc.Bacc(
        "TRN2", target_bir_lowering=False, debug=False, enable_asserts=False
    )
    src = nc.dram_tensor(
        "src", [B_PER_CORE, CP, H, W], mybir.dt.float32, kind="ExternalInput"
    ).ap()
    ihw = nc.dram_tensor(
        "ihw", [1, 2 * B_PER_CORE * N_CROPS], mybir.dt.int32, kind="ExternalInput"
    ).ap()
    out = nc.dram_tensor(
        "out",
        [B_PER_CORE * N_CROPS, CP, CROP, CROP],
        mybir.dt.float32,
        kind="ExternalOutput",
    ).ap()

    with nc.semaphore("done0") as s0, nc.semaphore("done1") as s1:
        for k in range(B_PER_CORE * N_CROPS):
            b = k // N_CROPS
            eng, dma_eng, sem = (
                (mybir.EngineType.SP, nc.sync, s0)
                if k % 2 == 0
                else (mybir.EngineType.Activation, nc.scalar, s1)
            )
            _, (h0, w0) = nc.values_load_multi_w_load_instructions(
                ihw[0:1, 2 * k:2 * k + 2], engines=(eng,),
                min_val=0, max_val=MAX_OFF, skip_runtime_bounds_check=True,
            )
            dma_eng.dma_start(
                out[k], src[b, :, ds(h0, CROP), ds(w0, CROP)]
            ).then_inc(sem, 16)

        nc.sync.wait_ge(s0, 16 * (B_PER_CORE * N_CROPS // 2))
        nc.sync.wait_ge(s1, 16 * (B_PER_CORE * N_CROPS // 2))

    nc.compile()
    return nc


def _get_program():
    global _PROGRAM
    if _PROGRAM is None:
        _PROGRAM = _build_program()
    return _PROGRAM


def _pos_planes():
    yy, xx = np.meshgrid(
        np.arange(H, dtype=np.float32) / H,
        np.arange(W, dtype=np.float32) / W,
        indexing="ij",
    )
    return np.stack((yy, xx))


def make_in_maps(images, crop_inds_h, crop_inds_w):
    pos = np.broadcast_to(_pos_planes()[None], (B_PER_CORE, 2, H, W))
    in_maps = []
    for c in range(N_CORES):
        sl = slice(c * B_PER_CORE, (c + 1) * B_PER_CORE)
        src = np.ascontiguousarray(
            np.concatenate(
                (np.asarray(images[sl], dtype=np.float32), pos), axis=1
            )
        )
        ihw = np.stack(
            (
                np.asarray(crop_inds_h[sl], dtype=np.int32).reshape(-1),
                np.asarray(crop_inds_w[sl], dtype=np.int32).reshape(-1),
            ),
            axis=1,
        ).reshape(1, -1)
        in_maps.append({"src": src, "ihw": np.ascontiguousarray(ihw)})
    return in_maps


def kernel(images, crop_inds_h, crop_inds_w):
    nc = _get_program()
    in_maps = make_in_maps(images, crop_inds_h, crop_inds_w)
    res = run_bass_kernel_spmd(nc, in_maps, core_ids=list(range(N_CORES)))
    return np.concatenate([r["out"] for r in res.results], axis=0)
